# revision 12
# baseline (speedup 1.0000x reference)
"""Mean-aggregator kernel for Trainium2 (Bass/Tile), 8-core SPMD.

mailbox: [50000, 32, 128] f32  ->  out: [50000, 128] f32 = mean over axis 1.

Sharding: node axis (dim 0) split evenly across 8 cores (6250 nodes/core).
Per core: DMA-bound streaming reduction.
  - load [128 part, k nodes * 32 deg * 128 feat] contiguous tiles (4 MB steady)
  - in-place halving tree over the deg axis: big levels on DVE (1 elem/cyc for
    contiguous APs), the two smallest levels on the otherwise-idle GpSimd so
    DVE stays below the per-tile DMA time and never throttles the pipeline
  - ACT scales by 1/32 and the store goes out on the ACT HWDGE ring
"""

import numpy as np

import concourse.bass as bass
import concourse.tile as tile
from concourse import bacc, mybir
from concourse.bass_utils import run_bass_kernel_spmd

N_CORES = 8
N_NODES = 50000
PER_CORE = N_NODES // N_CORES  # 6250
DEG = 32
FEAT = 128

N_LEVELS = 5  # log2(DEG)
GP_LEVELS = 0  # smallest tree levels routed to GpSimd (0: GpSimd TT is slow under DMA load)
SCALE = 1.0 / DEG


def emit_block(nc, in_pool, out_pool, mail, out, n0, nnodes, kpp):
    """One pipeline block: nnodes starting at n0, kpp nodes per partition."""
    P = nnodes // kpp
    assert P * kpp == nnodes and P <= 128
    t = in_pool.tile([P, kpp * DEG * FEAT], mybir.dt.float32, tag="t")
    src = mail[n0 : n0 + nnodes].rearrange("(p k) d f -> p (k d f)", p=P)
    nc.sync.dma_start(out=t[:], in_=src)

    v = t[:].rearrange("p (k d f) -> p k d f", k=kpp, d=DEG, f=FEAT)
    h = DEG
    level = 0
    while h > 1:
        h //= 2
        eng = nc.vector if level < N_LEVELS - GP_LEVELS else nc.gpsimd
        eng.tensor_add(v[:, :, 0:h, :], v[:, :, 0:h, :], v[:, :, h : 2 * h, :])
        level += 1

    o = out_pool.tile([P, kpp * FEAT], mybir.dt.float32, tag="o")
    nc.scalar.mul(o[:].rearrange("p (k f) -> p k f", k=kpp), v[:, :, 0, :], SCALE)
    dst = out[n0 : n0 + nnodes].rearrange("(p k) f -> p (k f)", p=P)
    nc.scalar.dma_start(out=dst, in_=o[:])


def build():
    nc = bacc.Bacc()
    mail = nc.dram_tensor(
        "mailbox", [PER_CORE, DEG, FEAT], mybir.dt.float32, kind="ExternalInput"
    )
    out = nc.dram_tensor(
        "out", [PER_CORE, FEAT], mybir.dt.float32, kind="ExternalOutput"
    )

    with tile.TileContext(nc) as tc:
        with (
            tc.tile_pool(name="inp", bufs=4) as in_pool,
            tc.tile_pool(name="outp", bufs=3) as out_pool,
        ):
            blocks = []
            # Ramp: two 128-node tiles so compute starts ~4x sooner than
            # waiting for a full 4 MB tile.
            n0 = 0
            for _ in range(2):
                blocks.append((n0, 128, 1))
                n0 += 128
            while n0 + 256 <= PER_CORE:
                blocks.append((n0, 256, 2))
                n0 += 256
            rem = PER_CORE - n0
            if rem:
                blocks.append((n0, rem, 1))

            for bn0, bn, kpp in blocks:
                emit_block(nc, in_pool, out_pool, mail, out, bn0, bn, kpp)

    if not nc.is_finalized():
        nc.finalize()
    return nc


_NC_CACHE = None


def _get_nc():
    global _NC_CACHE
    if _NC_CACHE is None:
        _NC_CACHE = build()
    return _NC_CACHE


def run(mailbox: np.ndarray, trace: bool = False, **trace_kwargs):
    mailbox = np.ascontiguousarray(np.asarray(mailbox, dtype=np.float32))
    assert mailbox.shape == (N_NODES, DEG, FEAT), mailbox.shape
    nc = _get_nc()
    shards = mailbox.reshape(N_CORES, PER_CORE, DEG, FEAT)
    in_maps = [{"mailbox": shards[i]} for i in range(N_CORES)]
    res = run_bass_kernel_spmd(
        nc, in_maps, list(range(N_CORES)), trace=trace, **trace_kwargs
    )
    full = np.concatenate([res.results[i]["out"] for i in range(N_CORES)], axis=0)
    return full, res


def kernel(mailbox: np.ndarray) -> np.ndarray:
    full, _ = run(mailbox, trace=False)
    return full


# revision 13
# speedup vs baseline: 1.0302x; 1.0302x over previous
"""Mean-aggregator kernel for Trainium2 (Bass/Tile), 8-core SPMD.

mailbox: [50000, 32, 128] f32  ->  out: [50000, 128] f32 = mean over axis 1.

Sharding: node axis (dim 0) split evenly across 8 cores (6250 nodes/core).
Per core: DMA-bound streaming reduction.
  - load [128 part, k nodes * 32 deg * 128 feat] contiguous tiles (4 MB steady)
  - in-place halving tree over the deg axis: big levels on DVE (1 elem/cyc for
    contiguous APs), the two smallest levels on the otherwise-idle GpSimd so
    DVE stays below the per-tile DMA time and never throttles the pipeline
  - ACT scales by 1/32 and the store goes out on the ACT HWDGE ring
"""

import numpy as np

import concourse.bass as bass
import concourse.tile as tile
from concourse import bacc, mybir
from concourse.bass_utils import run_bass_kernel_spmd

N_CORES = 8
N_NODES = 50000
PER_CORE = N_NODES // N_CORES  # 6250
DEG = 32
FEAT = 128

N_LEVELS = 5  # log2(DEG)
GP_LEVELS = 0  # smallest tree levels routed to GpSimd (0: GpSimd TT is slow under DMA load)
SCALE = 1.0 / DEG


def emit_block(nc, in_pool, out_pool, mail, out, n0, nnodes, kpp):
    """One pipeline block: nnodes starting at n0, kpp nodes per partition."""
    P = nnodes // kpp
    assert P * kpp == nnodes and P <= 128
    t = in_pool.tile([P, kpp * DEG * FEAT], mybir.dt.float32, tag="t")
    src = mail[n0 : n0 + nnodes].rearrange("(p k) d f -> p (k d f)", p=P)
    nc.gpsimd.dma_start(out=t[:], in_=src)

    v = t[:].rearrange("p (k d f) -> p k d f", k=kpp, d=DEG, f=FEAT)
    h = DEG
    level = 0
    while h > 1:
        h //= 2
        eng = nc.vector if level < N_LEVELS - GP_LEVELS else nc.gpsimd
        eng.tensor_add(v[:, :, 0:h, :], v[:, :, 0:h, :], v[:, :, h : 2 * h, :])
        level += 1

    o = out_pool.tile([P, kpp * FEAT], mybir.dt.float32, tag="o")
    nc.scalar.mul(o[:].rearrange("p (k f) -> p k f", k=kpp), v[:, :, 0, :], SCALE)
    dst = out[n0 : n0 + nnodes].rearrange("(p k) f -> p (k f)", p=P)
    nc.scalar.dma_start(out=dst, in_=o[:])


def build():
    nc = bacc.Bacc()
    mail = nc.dram_tensor(
        "mailbox", [PER_CORE, DEG, FEAT], mybir.dt.float32, kind="ExternalInput"
    )
    out = nc.dram_tensor(
        "out", [PER_CORE, FEAT], mybir.dt.float32, kind="ExternalOutput"
    )

    with tile.TileContext(nc) as tc:
        with (
            tc.tile_pool(name="inp", bufs=4) as in_pool,
            tc.tile_pool(name="outp", bufs=3) as out_pool,
        ):
            blocks = []
            # Ramp: two 128-node tiles so compute starts ~4x sooner than
            # waiting for a full 4 MB tile.
            n0 = 0
            for _ in range(2):
                blocks.append((n0, 128, 1))
                n0 += 128
            while n0 + 256 <= PER_CORE:
                blocks.append((n0, 256, 2))
                n0 += 256
            rem = PER_CORE - n0
            if rem:
                blocks.append((n0, rem, 1))

            for bn0, bn, kpp in blocks:
                emit_block(nc, in_pool, out_pool, mail, out, bn0, bn, kpp)

    if not nc.is_finalized():
        nc.finalize()
    return nc


_NC_CACHE = None


def _get_nc():
    global _NC_CACHE
    if _NC_CACHE is None:
        _NC_CACHE = build()
    return _NC_CACHE


def run(mailbox: np.ndarray, trace: bool = False, **trace_kwargs):
    mailbox = np.ascontiguousarray(np.asarray(mailbox, dtype=np.float32))
    assert mailbox.shape == (N_NODES, DEG, FEAT), mailbox.shape
    nc = _get_nc()
    shards = mailbox.reshape(N_CORES, PER_CORE, DEG, FEAT)
    in_maps = [{"mailbox": shards[i]} for i in range(N_CORES)]
    res = run_bass_kernel_spmd(
        nc, in_maps, list(range(N_CORES)), trace=trace, **trace_kwargs
    )
    full = np.concatenate([res.results[i]["out"] for i in range(N_CORES)], axis=0)
    return full, res


def kernel(mailbox: np.ndarray) -> np.ndarray:
    full, _ = run(mailbox, trace=False)
    return full


# revision 15
# speedup vs baseline: 1.0456x; 1.0149x over previous
"""Mean-aggregator kernel for Trainium2 (Bass/Tile), 8-core SPMD.

mailbox: [50000, 32, 128] f32  ->  out: [50000, 128] f32 = mean over axis 1.

Sharding: node axis (dim 0) split evenly across 8 cores (6250 nodes/core).
Per core: DMA-bound streaming reduction.
  - load [128 part, k nodes * 32 deg * 128 feat] contiguous tiles (4 MB steady)
  - in-place halving tree over the deg axis: big levels on DVE (1 elem/cyc for
    contiguous APs), the two smallest levels on the otherwise-idle GpSimd so
    DVE stays below the per-tile DMA time and never throttles the pipeline
  - ACT scales by 1/32 and the store goes out on the ACT HWDGE ring
"""

import numpy as np

import concourse.bass as bass
import concourse.tile as tile
from concourse import bacc, mybir
from concourse.bass_utils import run_bass_kernel_spmd

N_CORES = 8
N_NODES = 50000
PER_CORE = N_NODES // N_CORES  # 6250
DEG = 32
FEAT = 128

N_LEVELS = 5  # log2(DEG)
GP_LEVELS = 0  # smallest tree levels routed to GpSimd (0: GpSimd TT is slow under DMA load)
SCALE = 1.0 / DEG


def emit_block(nc, in_pool, out_pool, mail, out, n0, nnodes, kpp):
    """One pipeline block: nnodes starting at n0, kpp nodes per partition."""
    P = nnodes // kpp
    assert P * kpp == nnodes and P <= 128
    t = in_pool.tile([P, kpp * DEG * FEAT], mybir.dt.float32, tag="t")
    src = mail[n0 : n0 + nnodes].rearrange("(p k) d f -> p (k d f)", p=P)
    nc.gpsimd.dma_start(out=t[:], in_=src)

    v = t[:].rearrange("p (k d f) -> p k d f", k=kpp, d=DEG, f=FEAT)
    h = DEG
    level = 0
    while h > 1:
        h //= 2
        eng = nc.vector if level < N_LEVELS - GP_LEVELS else nc.gpsimd
        eng.tensor_add(v[:, :, 0:h, :], v[:, :, 0:h, :], v[:, :, h : 2 * h, :])
        level += 1

    o = out_pool.tile([P, kpp * FEAT], mybir.dt.float32, tag="o")
    nc.scalar.mul(o[:].rearrange("p (k f) -> p k f", k=kpp), v[:, :, 0, :], SCALE)
    dst = out[n0 : n0 + nnodes].rearrange("(p k) f -> p (k f)", p=P)
    nc.scalar.dma_start(out=dst, in_=o[:])


def build():
    nc = bacc.Bacc()
    mail = nc.dram_tensor(
        "mailbox", [PER_CORE, DEG, FEAT], mybir.dt.float32, kind="ExternalInput"
    )
    out = nc.dram_tensor(
        "out", [PER_CORE, FEAT], mybir.dt.float32, kind="ExternalOutput"
    )

    with tile.TileContext(nc) as tc:
        with (
            tc.tile_pool(name="inp", bufs=5) as in_pool,
            tc.tile_pool(name="outp", bufs=3) as out_pool,
        ):
            blocks = []
            # Ramp: two 128-node tiles so compute starts ~4x sooner than
            # waiting for a full 4 MB tile.
            n0 = 0
            for _ in range(4):
                blocks.append((n0, 128, 1))
                n0 += 128
            while n0 + 256 <= PER_CORE:
                blocks.append((n0, 256, 2))
                n0 += 256
            rem = PER_CORE - n0
            if rem:
                blocks.append((n0, rem, 1))

            for bn0, bn, kpp in blocks:
                emit_block(nc, in_pool, out_pool, mail, out, bn0, bn, kpp)

    if not nc.is_finalized():
        nc.finalize()
    return nc


_NC_CACHE = None


def _get_nc():
    global _NC_CACHE
    if _NC_CACHE is None:
        _NC_CACHE = build()
    return _NC_CACHE


def run(mailbox: np.ndarray, trace: bool = False, **trace_kwargs):
    mailbox = np.ascontiguousarray(np.asarray(mailbox, dtype=np.float32))
    assert mailbox.shape == (N_NODES, DEG, FEAT), mailbox.shape
    nc = _get_nc()
    shards = mailbox.reshape(N_CORES, PER_CORE, DEG, FEAT)
    in_maps = [{"mailbox": shards[i]} for i in range(N_CORES)]
    res = run_bass_kernel_spmd(
        nc, in_maps, list(range(N_CORES)), trace=trace, **trace_kwargs
    )
    full = np.concatenate([res.results[i]["out"] for i in range(N_CORES)], axis=0)
    return full, res


def kernel(mailbox: np.ndarray) -> np.ndarray:
    full, _ = run(mailbox, trace=False)
    return full


# revision 17
# speedup vs baseline: 1.0959x; 1.0481x over previous
"""Mean-aggregator kernel for Trainium2 (Bass/Tile), 8-core SPMD.

mailbox: [50000, 32, 128] f32  ->  out: [50000, 128] f32 = mean over axis 1.

Sharding: node axis (dim 0) split evenly across 8 cores (6250 nodes/core).
Per core: DMA-bound streaming reduction.
  - load [128 part, k nodes * 32 deg * 128 feat] contiguous tiles (4 MB steady)
  - in-place halving tree over the deg axis: big levels on DVE (1 elem/cyc for
    contiguous APs), the two smallest levels on the otherwise-idle GpSimd so
    DVE stays below the per-tile DMA time and never throttles the pipeline
  - ACT scales by 1/32 and the store goes out on the ACT HWDGE ring
"""

import numpy as np

import concourse.bass as bass
import concourse.tile as tile
from concourse import bacc, mybir
from concourse.bass_utils import run_bass_kernel_spmd

N_CORES = 8
N_NODES = 50000
PER_CORE = N_NODES // N_CORES  # 6250
DEG = 32
FEAT = 128

N_LEVELS = 5  # log2(DEG)
GP_LEVELS = 0  # smallest tree levels routed to GpSimd (0: GpSimd TT is slow under DMA load)
SCALE = 1.0 / DEG


def emit_block(nc, in_pool, out_pool, mail, out, n0, nnodes, kpp, load_engine=None):
    """One pipeline block: nnodes starting at n0, kpp nodes per partition."""
    P = nnodes // kpp
    assert P * kpp == nnodes and P <= 128
    t = in_pool.tile([P, kpp * DEG * FEAT], mybir.dt.float32, tag="t")
    src = mail[n0 : n0 + nnodes].rearrange("(p k) d f -> p (k d f)", p=P)
    (load_engine or nc.gpsimd).dma_start(out=t[:], in_=src)

    v = t[:].rearrange("p (k d f) -> p k d f", k=kpp, d=DEG, f=FEAT)
    h = DEG
    level = 0
    while h > 1:
        h //= 2
        eng = nc.vector if level < N_LEVELS - GP_LEVELS else nc.gpsimd
        eng.tensor_add(v[:, :, 0:h, :], v[:, :, 0:h, :], v[:, :, h : 2 * h, :])
        level += 1

    o = out_pool.tile([P, kpp * FEAT], mybir.dt.float32, tag="o")
    nc.scalar.mul(o[:].rearrange("p (k f) -> p k f", k=kpp), v[:, :, 0, :], SCALE)
    dst = out[n0 : n0 + nnodes].rearrange("(p k) f -> p (k f)", p=P)
    nc.scalar.dma_start(out=dst, in_=o[:])


def build():
    nc = bacc.Bacc()
    mail = nc.dram_tensor(
        "mailbox", [PER_CORE, DEG, FEAT], mybir.dt.float32, kind="ExternalInput"
    )
    out = nc.dram_tensor(
        "out", [PER_CORE, FEAT], mybir.dt.float32, kind="ExternalOutput"
    )

    with tile.TileContext(nc) as tc:
        with (
            tc.tile_pool(name="inp", bufs=5) as in_pool,
            tc.tile_pool(name="outp", bufs=3) as out_pool,
        ):
            blocks = []
            # Ramp: two 128-node tiles so compute starts ~4x sooner than
            # waiting for a full 4 MB tile.
            n0 = 0
            for _ in range(4):
                blocks.append((n0, 128, 1))
                n0 += 128
            while n0 + 256 <= PER_CORE:
                blocks.append((n0, 256, 2))
                n0 += 256
            rem = PER_CORE - n0
            if rem:
                blocks.append((n0, rem, 1))

            for bi, (bn0, bn, kpp) in enumerate(blocks):
                # First (wait-free) ramp loads go out on the idle sync HWDGE
                # ring: ~1.5 us lower first-byte latency than the Q7 SWDGE
                # path, so the pipeline fills sooner.
                eng = nc.sync if bi < 4 else None
                emit_block(nc, in_pool, out_pool, mail, out, bn0, bn, kpp, eng)

    if not nc.is_finalized():
        nc.finalize()
    return nc


_NC_CACHE = None


def _get_nc():
    global _NC_CACHE
    if _NC_CACHE is None:
        _NC_CACHE = build()
    return _NC_CACHE


def run(mailbox: np.ndarray, trace: bool = False, **trace_kwargs):
    mailbox = np.ascontiguousarray(np.asarray(mailbox, dtype=np.float32))
    assert mailbox.shape == (N_NODES, DEG, FEAT), mailbox.shape
    nc = _get_nc()
    shards = mailbox.reshape(N_CORES, PER_CORE, DEG, FEAT)
    in_maps = [{"mailbox": shards[i]} for i in range(N_CORES)]
    res = run_bass_kernel_spmd(
        nc, in_maps, list(range(N_CORES)), trace=trace, **trace_kwargs
    )
    full = np.concatenate([res.results[i]["out"] for i in range(N_CORES)], axis=0)
    return full, res


def kernel(mailbox: np.ndarray) -> np.ndarray:
    full, _ = run(mailbox, trace=False)
    return full


# revision 18
# speedup vs baseline: 1.1182x; 1.0204x over previous
"""Mean-aggregator kernel for Trainium2 (Bass/Tile), 8-core SPMD.

mailbox: [50000, 32, 128] f32  ->  out: [50000, 128] f32 = mean over axis 1.

Sharding: node axis (dim 0) split evenly across 8 cores (6250 nodes/core).
Per core: DMA-bound streaming reduction.
  - load [128 part, k nodes * 32 deg * 128 feat] contiguous tiles (4 MB steady)
  - in-place halving tree over the deg axis: big levels on DVE (1 elem/cyc for
    contiguous APs), the two smallest levels on the otherwise-idle GpSimd so
    DVE stays below the per-tile DMA time and never throttles the pipeline
  - ACT scales by 1/32 and the store goes out on the ACT HWDGE ring
"""

import numpy as np

import concourse.bass as bass
import concourse.tile as tile
from concourse import bacc, mybir
from concourse.bass_utils import run_bass_kernel_spmd

N_CORES = 8
N_NODES = 50000
PER_CORE = N_NODES // N_CORES  # 6250
DEG = 32
FEAT = 128

N_LEVELS = 5  # log2(DEG)
GP_LEVELS = 0  # smallest tree levels routed to GpSimd (0: GpSimd TT is slow under DMA load)
SCALE = 1.0 / DEG


def emit_block(nc, in_pool, out_pool, mail, out, n0, nnodes, kpp, load_engine=None):
    """One pipeline block: nnodes starting at n0, kpp nodes per partition."""
    P = nnodes // kpp
    assert P * kpp == nnodes and P <= 128
    t = in_pool.tile([P, kpp * DEG * FEAT], mybir.dt.float32, tag="t")
    src = mail[n0 : n0 + nnodes].rearrange("(p k) d f -> p (k d f)", p=P)
    (load_engine or nc.gpsimd).dma_start(out=t[:], in_=src)

    v = t[:].rearrange("p (k d f) -> p k d f", k=kpp, d=DEG, f=FEAT)
    h = DEG
    level = 0
    while h > 1:
        h //= 2
        eng = nc.vector if level < N_LEVELS - GP_LEVELS else nc.gpsimd
        eng.tensor_add(v[:, :, 0:h, :], v[:, :, 0:h, :], v[:, :, h : 2 * h, :])
        level += 1

    o = out_pool.tile([P, kpp * FEAT], mybir.dt.float32, tag="o")
    nc.scalar.mul(o[:].rearrange("p (k f) -> p k f", k=kpp), v[:, :, 0, :], SCALE)
    dst = out[n0 : n0 + nnodes].rearrange("(p k) f -> p (k f)", p=P)
    nc.scalar.dma_start(out=dst, in_=o[:])


def build():
    nc = bacc.Bacc()
    mail = nc.dram_tensor(
        "mailbox", [PER_CORE, DEG, FEAT], mybir.dt.float32, kind="ExternalInput"
    )
    out = nc.dram_tensor(
        "out", [PER_CORE, FEAT], mybir.dt.float32, kind="ExternalOutput"
    )

    with tile.TileContext(nc) as tc:
        with (
            tc.tile_pool(name="inp", bufs=5) as in_pool,
            tc.tile_pool(name="outp", bufs=3) as out_pool,
        ):
            blocks = []
            # Ramp: two 128-node tiles so compute starts ~4x sooner than
            # waiting for a full 4 MB tile.
            n0 = 0
            for _ in range(4):
                blocks.append((n0, 128, 1))
                n0 += 128
            while n0 + 256 <= PER_CORE:
                blocks.append((n0, 256, 2))
                n0 += 256
            rem = PER_CORE - n0
            if rem:
                blocks.append((n0, rem, 1))

            for bi, (bn0, bn, kpp) in enumerate(blocks):
                emit_block(nc, in_pool, out_pool, mail, out, bn0, bn, kpp)

    if not nc.is_finalized():
        nc.finalize()
    return nc


_NC_CACHE = None


def _get_nc():
    global _NC_CACHE
    if _NC_CACHE is None:
        _NC_CACHE = build()
    return _NC_CACHE


def run(mailbox: np.ndarray, trace: bool = False, **trace_kwargs):
    mailbox = np.ascontiguousarray(np.asarray(mailbox, dtype=np.float32))
    assert mailbox.shape == (N_NODES, DEG, FEAT), mailbox.shape
    nc = _get_nc()
    shards = mailbox.reshape(N_CORES, PER_CORE, DEG, FEAT)
    in_maps = [{"mailbox": shards[i]} for i in range(N_CORES)]
    res = run_bass_kernel_spmd(
        nc, in_maps, list(range(N_CORES)), trace=trace, **trace_kwargs
    )
    full = np.concatenate([res.results[i]["out"] for i in range(N_CORES)], axis=0)
    return full, res


def kernel(mailbox: np.ndarray) -> np.ndarray:
    full, _ = run(mailbox, trace=False)
    return full
